# revision 1
# baseline (speedup 1.0000x reference)
"""Multi-head self-attention (B=2, S=2048, D=1024, H=16) on 8 Trainium2 NeuronCores.

Sharding: batch x head-group. Core c = b*4 + g handles batch b and heads 4g..4g+3
(Megatron-style TP: Wq/Wk/Wv column-sharded, Wo row-sharded; partial outputs
summed on the host).

Per-core kernel layout ("T-layout": sequence on the free dim everywhere):
  inputs (host-prepared):  xt [1024, 2048] = x[b].T;  wq/wk/wv [1024, 256]
  (scale-folded, transposed);  wo [256, 1024] (scale-folded, transposed)
  QT/KT = (w.T @ xt) [256, 2048]        d' on partitions, heads pair-stacked
  V     = (xt.T @ wv) [2048, 260]       natural layout + ones column per head
  scoresT[k, q] = KT_h-slices.T @ QT_h  per head, k on partitions
  expT = exp(scoresT / 8)               (no max subtraction: |scores| <~ 2)
  ctxT_aug[d+1, q] = [V_h | 1].T @ expT accumulated over k-chunks; row 64 = denom
  ctxT = ctxT_aug[0:64] * (1/denom)     denominator broadcast via gpsimd
  outT_partial = wo.T @ ctxT [1024, 2048]
Host: out[b] = sum_g outT[b, g].T

Every matmul uses K<=64 contraction (row-tiled 64x128 PE mode, tiles T0/T8
run concurrently) so the PE never switches tiling modes.
"""
import sys

sys.path.insert(0, "/opt/trn_rl_repo")

import numpy as np

import concourse.bass as bass
import concourse.tile as tile
from concourse import bacc, mybir
from concourse.bass_utils import run_bass_kernel_spmd

F32 = mybir.dt.float32
MM_DT = mybir.dt.float32r  # 1 cycle/row at N>=256 (fp32 is 4); fp32 storage

S = 2048          # sequence length per batch
D = 1024          # embedding dim
HG = 4            # heads per core
HD = 64           # head dim
GC = HG * HD      # group cols = 256
P = 128
NQ = 4            # q chunks of 512
QW = 512          # q chunk width
NKC = 16          # key-position chunks of 128
KO = 8            # contraction chunks of 128 over D
VW = HD + 1       # V columns per head incl. ones column

_NC_CACHE = {}
DEBUG_DUMPS = False


def _build():
    if "nc" in _NC_CACHE:
        return _NC_CACHE["nc"]
    nc = bacc.Bacc(trn_type="TRN2", target_bir_lowering=False, debug=False)
    xt_d = nc.dram_tensor("xt", [D, S], MM_DT, kind="ExternalInput")
    wq_d = nc.dram_tensor("wq", [D, GC], MM_DT, kind="ExternalInput")
    wk_d = nc.dram_tensor("wk", [D, GC], MM_DT, kind="ExternalInput")
    wv_d = nc.dram_tensor("wv", [D, GC], MM_DT, kind="ExternalInput")
    wo_d = nc.dram_tensor("wo", [GC, D], MM_DT, kind="ExternalInput")
    out_d = nc.dram_tensor("out_t", [D, S], F32, kind="ExternalOutput")
    dbg = None
    if DEBUG_DUMPS:
        dbg = {
            "dbg_qt": nc.dram_tensor("dbg_qt", [P, 2, S], MM_DT, kind="ExternalOutput"),
            "dbg_kt": nc.dram_tensor("dbg_kt", [P, 2, S], MM_DT, kind="ExternalOutput"),
            "dbg_va": nc.dram_tensor("dbg_va", [P, NKC, HG * VW], MM_DT,
                                     kind="ExternalOutput"),
            "dbg_ct": nc.dram_tensor("dbg_ct", [P, 2, S], MM_DT, kind="ExternalOutput"),
            "dbg_ex": nc.dram_tensor("dbg_ex", [P, 4, QW], MM_DT, kind="ExternalOutput"),
            "dbg_sc": nc.dram_tensor("dbg_sc", [P, 4, QW], F32, kind="ExternalOutput"),
        }

    scr_d = nc.dram_tensor("nrm_scratch", [2, NQ, 2, QW], F32)
    with tile.TileContext(nc) as tc:
        _emit(nc, tc, xt_d, wq_d, wk_d, wv_d, wo_d, out_d, scr_d, dbg)
    nc.compile()
    _NC_CACHE["nc"] = nc
    return nc


def _emit(nc, tc, xt_d, wq_d, wk_d, wv_d, wo_d, out_d, scr_d, dbg=None):
    with tc.tile_pool(name="big", bufs=1) as big:
        # ---- persistent SBUF tensors (~96KB/partition) ----
        wo_sb = big.tile([P, 2, D], MM_DT)        # [d'(128) x chunk x e]
        qt = big.tile([P, 2, S], MM_DT)           # QT: head h at parts (h%2)*64, chunk h//2
        kt = big.tile([P, 2, S], MM_DT)
        va = big.tile([P, NKC, HG * VW], MM_DT)   # V natural + ones col per head
        ct = big.tile([P, 2, S], MM_DT)           # ctxT, same head layout as qt

        nc.sync.dma_start(wo_sb[:], wo_d.rearrange("(c p) e -> p c e", p=P))

        # ones columns of V_aug (col HD of each VW-wide head block)
        va_h = va[:].rearrange("p s (h c) -> p s h c", c=VW)
        for h in range(HG):
            # fp32 1.0 bit pattern; walrus memset rejects float32r directly
            nc.vector.memset(
                va_h[:, :, h, HD:HD + 1].bitcast(mybir.dt.uint32), 0x3F800000)

        def mm_pair(pa, pb, lhsT, rhs, start, stop):
            """Row-tiled K=64 pair: T0 (parts 0-63) -> pa, T8 (parts 64-127) -> pb."""
            nc.tensor.matmul(pa, lhsT[0:64], rhs[0:64], start=start, stop=stop)
            nc.tensor.matmul(pb, lhsT[64:128], rhs[64:128], start=start, stop=stop)

        # ================= phase 1: projections =================
        with tc.tile_pool(name="xw", bufs=1) as xw, \
             tc.tile_pool(name="evac", bufs=3) as evac, \
             tc.tile_pool(name="ps_proj", bufs=4, space="PSUM") as ps_proj:
            xs = xw.tile([P, KO, S], MM_DT)       # x.T, [d_in(128) x ko x s]
            wq = xw.tile([P, KO, GC], MM_DT)
            wk = xw.tile([P, KO, GC], MM_DT)
            wv = xw.tile([P, KO, GC], MM_DT)
            for ko in range(KO):
                nc.sync.dma_start(xs[:, ko, :], xt_d[ko * P:(ko + 1) * P, :])
            nc.sync.dma_start(wq[:], wq_d.rearrange("(ko p) m -> p ko m", p=P))
            nc.sync.dma_start(wk[:], wk_d.rearrange("(ko p) m -> p ko m", p=P))
            nc.sync.dma_start(wv[:], wv_d.rearrange("(ko p) m -> p ko m", p=P))

            # QT/KT: transposed out [d' x s]
            for w_sb, dst in ((wq, qt), (wk, kt)):
                for m in range(2):          # d' chunk = head pair
                    for n in range(NQ):
                        pa = ps_proj.tile([P, QW], F32, tag="pp")
                        pb = ps_proj.tile([P, QW], F32, tag="pp")
                        for ko in range(KO):
                            mm_pair(pa[:], pb[:],
                                    w_sb[:, ko, m * P:(m + 1) * P],
                                    xs[:, ko, n * QW:(n + 1) * QW],
                                    start=(ko == 0), stop=(ko == KO - 1))
                        t = evac.tile([P, QW], F32, tag="ev")
                        nc.vector.tensor_copy(t[:], pb[:])
                        nc.vector.tensor_tensor(
                            dst[:, m, n * QW:(n + 1) * QW],
                            pa[:], t[:], mybir.AluOpType.add)

            # V natural: [s(128) x 256] per s-chunk
            for sc in range(NKC):
                pa = ps_proj.tile([P, QW], F32, tag="pp")
                pb = ps_proj.tile([P, QW], F32, tag="pp")
                for ko in range(KO):
                    mm_pair(pa[:, :GC], pb[:, :GC],
                            xs[:, ko, sc * P:(sc + 1) * P],
                            wv[:, ko, :],
                            start=(ko == 0), stop=(ko == KO - 1))
                tv = evac.tile([P, QW], F32, tag="ev")
                nc.vector.tensor_copy(tv[:, :GC], pb[:, :GC])
                nc.vector.tensor_tensor(
                    va_h[:, sc, :, 0:HD],
                    pa[:, :GC].rearrange("p (h c) -> p h c", c=HD),
                    tv[:, :GC].rearrange("p (h c) -> p h c", c=HD),
                    mybir.AluOpType.add)

        if dbg is not None:
            nc.sync.dma_start(dbg["dbg_qt"][:], qt[:])
            nc.sync.dma_start(dbg["dbg_kt"][:], kt[:])
            nc.sync.dma_start(dbg["dbg_va"][:], va[:])

        # ================= phase 2: attention =================
        with tc.tile_pool(name="expp", bufs=2) as expp, \
             tc.tile_pool(name="norm", bufs=2) as norm, \
             tc.tile_pool(name="ps_sc", bufs=1, space="PSUM") as ps_sc, \
             tc.tile_pool(name="ps_ctx", bufs=1, space="PSUM") as ps_ctx:
            for hp in range(2):         # head pair (even = parts 0-63, odd = 64-127)
                for n in range(NQ):
                    # ctx accumulators: [even/odd head] x [k-low/k-high half]
                    cps = [[ps_ctx.tile([P, QW], F32, tag=f"pc{e}{l}",
                                        name=f"pc{e}{l}_{hp}_{n}")
                            for l in range(2)] for e in range(2)]
                    for kb in range(NKC // 2):
                        sp = ps_sc.tile([P, 4, QW], F32, tag="psc")   # 4 banks
                        ex = expp.tile([P, 4, QW], MM_DT, tag="pex")
                        for j in range(4):
                            kc = kb * 2 + j // 2
                            lo = (j % 2) * 64
                            nc.tensor.matmul(
                                sp[:, j, :],
                                kt[lo:lo + 64, hp, kc * P:(kc + 1) * P],
                                qt[lo:lo + 64, hp, n * QW:(n + 1) * QW],
                                start=True, stop=True)
                        nc.scalar.activation(
                            ex[:].rearrange("p a b -> p (a b)"),
                            sp[:].rearrange("p a b -> p (a b)"),
                            mybir.ActivationFunctionType.Exp,
                            scale=0.125)
                        if dbg is not None and hp == 0 and n == 0 and kb == 0:
                            nc.sync.dma_start(dbg["dbg_ex"][:], ex[:])
                            spc = norm.tile([P, 4, QW], F32, tag="spdump")
                            nc.vector.tensor_copy(spc[:], sp[:])
                            nc.sync.dma_start(dbg["dbg_sc"][:], spc[:])
                        for j in range(4):
                            kc = kb * 2 + j // 2
                            e = j % 2
                            h = 2 * hp + e
                            for l in range(2):   # k-low / k-high 64-halves
                                nc.tensor.matmul(
                                    cps[e][l][0:VW, :],
                                    va[l * 64:(l + 1) * 64, kc, h * VW:(h + 1) * VW],
                                    ex[l * 64:(l + 1) * 64, j, :],
                                    start=(kb == 0 and j < 2),
                                    stop=(kb == NKC // 2 - 1 and j >= 2))
                    # normalize: ctxT = (A+B)[0:64] / (A+B)[64]
                    for e in range(2):
                        sm = norm.tile([P, QW], F32, tag="nsum")
                        bc = norm.tile([P, QW], F32, tag="nbc")
                        nc.vector.tensor_copy(sm[0:VW, :], cps[e][1][0:VW, :])
                        nc.vector.tensor_tensor(sm[0:VW, :], cps[e][0][0:VW, :],
                                                sm[0:VW, :], mybir.AluOpType.add)
                        nc.vector.reciprocal(sm[HD:VW, :], sm[HD:VW, :])
                        # partition-broadcast 1/denom via DRAM bounce
                        sl = scr_d[hp, n, e]
                        nc.sync.dma_start(sl.unsqueeze(0), sm[HD:VW, :])
                        bc_src = bass.AP(tensor=sl.tensor, offset=sl.offset,
                                         ap=[[0, 64]] + list(sl.ap))
                        nc.sync.dma_start(bc[0:64, :], bc_src)
                        nc.vector.tensor_tensor(
                            ct[e * 64:e * 64 + 64, hp, n * QW:(n + 1) * QW],
                            sm[0:HD, :], bc[0:64, :], mybir.AluOpType.mult)

        if dbg is not None:
            nc.sync.dma_start(dbg["dbg_ct"][:], ct[:])

        # ================= phase 3: output projection =================
        with tc.tile_pool(name="outp", bufs=3) as outp, \
             tc.tile_pool(name="ps_o", bufs=4, space="PSUM") as ps_o:
            for m in range(KO):         # e chunks of 128
                for n in range(NQ):
                    pa = ps_o.tile([P, QW], F32, tag="po")
                    pb = ps_o.tile([P, QW], F32, tag="po")
                    for c in range(2):
                        mm_pair(pa[:], pb[:],
                                wo_sb[:, c, m * P:(m + 1) * P],
                                ct[:, c, n * QW:(n + 1) * QW],
                                start=(c == 0), stop=(c == 1))
                    ot = outp.tile([P, QW], F32, tag="ot")
                    nc.vector.tensor_copy(ot[:], pb[:])
                    nc.vector.tensor_tensor(ot[:], pa[:], ot[:],
                                            mybir.AluOpType.add)
                    nc.sync.dma_start(
                        out_d[m * P:(m + 1) * P, n * QW:(n + 1) * QW], ot[:])


def _in_maps(x, wq_f, wk_f, wv_f, wo_f):
    maps = []
    for core in range(8):
        b, g = core // 4, core % 4
        cols = slice(g * GC, (g + 1) * GC)
        maps.append({
            "xt": np.ascontiguousarray(x[b].T),
            "wq": np.ascontiguousarray(wq_f[:, cols]),
            "wk": np.ascontiguousarray(wk_f[:, cols]),
            "wv": np.ascontiguousarray(wv_f[:, cols]),
            "wo": np.ascontiguousarray(wo_f[cols, :]),
        })
    return maps


def run_traced(x, Wq, Wk, Wv, Wo, q_scale, k_scale, v_scale, o_scale):
    """Like kernel() but with NTFF tracing; returns (out, exec_time_ns, trace_path)."""
    x = np.asarray(x, dtype=np.float32)
    wq_f = (np.asarray(Wq).T * np.asarray(q_scale).reshape(1, -1)).astype(np.float32)
    wk_f = (np.asarray(Wk).T * np.asarray(k_scale).reshape(1, -1)).astype(np.float32)
    wv_f = (np.asarray(Wv).T * np.asarray(v_scale).reshape(1, -1)).astype(np.float32)
    wo_f = (np.asarray(Wo).T * np.asarray(o_scale).reshape(1, -1)).astype(np.float32)
    nc = _build()
    res = run_bass_kernel_spmd(nc, _in_maps(x, wq_f, wk_f, wv_f, wo_f),
                               core_ids=list(range(8)), trace=True)
    out = np.zeros((x.shape[0], S, D), dtype=np.float32)
    for core in range(8):
        out[core // 4] += res.results[core]["out_t"].T
    trace_path = None
    if res.instructions_and_trace is not None:
        trace_path = res.instructions_and_trace[1]
    return out, res.exec_time_ns, trace_path


def kernel(x, Wq, Wk, Wv, Wo, q_scale, k_scale, v_scale, o_scale):
    B = x.shape[0]
    x = np.asarray(x, dtype=np.float32)
    wq_f = (np.asarray(Wq).T * np.asarray(q_scale).reshape(1, -1)).astype(np.float32)
    wk_f = (np.asarray(Wk).T * np.asarray(k_scale).reshape(1, -1)).astype(np.float32)
    wv_f = (np.asarray(Wv).T * np.asarray(v_scale).reshape(1, -1)).astype(np.float32)
    wo_f = (np.asarray(Wo).T * np.asarray(o_scale).reshape(1, -1)).astype(np.float32)

    nc = _build()
    res = run_bass_kernel_spmd(nc, _in_maps(x, wq_f, wk_f, wv_f, wo_f),
                               core_ids=list(range(8)))
    out = np.zeros((B, S, D), dtype=np.float32)
    for core in range(8):
        b = core // 4
        out[b] += res.results[core]["out_t"].T
    return out



# revision 11
# speedup vs baseline: 2.0523x; 2.0523x over previous
"""Multi-head self-attention (B=2, S=2048, D=1024, H=16) on 8 Trainium2 NeuronCores.

Sharding: batch x head-group. Core c = b*4 + g handles batch b and heads 4g..4g+3
(Megatron-style TP: Wq/Wk/Wv column-sharded, Wo row-sharded; partial outputs
summed on the host).

v2 design (bf16 compute, fp32 PSUM accumulation):
  T-layout: sequence on the free dim everywhere.
  inputs:  xt [1024, 2048] = x[b].T (bf16);  wq/wk/wv [1024, 256] scale-folded
  (bf16);  wo [256, 1024] (bf16)
  QT/KT = (w.T @ xt) [256, 2048]      d' on partitions, K=128 chains
  V     = (xt.T @ wv) [2048, 260]     natural layout + ones column per head
  scoresT[k, q] = KT_h.T @ QT_h       per head, K=64 row-tiled pairs (2 heads
                                      concurrent in rows 0-63 / 64-127)
  expT = exp(scoresT / 8)             bf16, scalar engine (no max subtraction:
                                      |scores| <~ 2)
  ctxT_aug[d'+1, q] = [V_h | 1].T @ expT   K=128 chains over k-chunks;
                                      row 64 = softmax denominator
  ctxT = ctxT_aug[0:64] * (1/denom)   recip_approx_fast + gpsimd
                                      partition_broadcast + DVE mult
  outT_partial = wo.T @ ctxT [1024, 2048]  fp32 out
Host: out[b] = sum_g outT[b, g].T

Emission order software-pipelines the phases: V-projection chains run inside
the first attention block's exp-wait gaps, Q-projections for q-block n+2 and
output-projection chains for q-block n-1 run inside later blocks, so the PE
never idles long enough for HAM to re-throttle.
"""
import sys

sys.path.insert(0, "/opt/trn_rl_repo")

import numpy as np
import ml_dtypes

import concourse.bass as bass
import concourse.tile as tile
from concourse import bacc, library_config, mybir
from concourse.bass_utils import run_bass_kernel_spmd

F32 = mybir.dt.float32
BF16 = mybir.dt.bfloat16

S = 2048          # sequence length per batch
D = 1024          # embedding dim
HG = 4            # heads per core
HD = 64           # head dim
GC = HG * HD      # group cols = 256
P = 128
NQ = 4            # q chunks of 512
QW = 512          # q chunk width
NKC = 16          # key-position chunks of 128
KO = 8            # contraction chunks of 128 over D
VW = HD + 1       # V columns per head incl. ones column

_NC_CACHE = {}
DEBUG_DUMPS = False


def _build():
    if "nc" in _NC_CACHE:
        return _NC_CACHE["nc"]
    nc = bacc.Bacc(trn_type="TRN2", target_bir_lowering=False, debug=False)
    xt_d = nc.dram_tensor("xt", [D, S], BF16, kind="ExternalInput")
    wq_d = nc.dram_tensor("wq", [D, GC], BF16, kind="ExternalInput")
    wk_d = nc.dram_tensor("wk", [D, GC], BF16, kind="ExternalInput")
    wv_d = nc.dram_tensor("wv", [D, GC], BF16, kind="ExternalInput")
    wo_d = nc.dram_tensor("wo", [GC, D], BF16, kind="ExternalInput")
    out_d = nc.dram_tensor("out_t", [D, S], F32, kind="ExternalOutput")
    dbg = None
    if DEBUG_DUMPS:
        dbg = {
            "dbg_kt": nc.dram_tensor("dbg_kt", [P, 2, S], BF16, kind="ExternalOutput"),
            "dbg_qt": nc.dram_tensor("dbg_qt", [P, 2, S], BF16, kind="ExternalOutput"),
            "dbg_ct": nc.dram_tensor("dbg_ct", [P, 2, S], BF16, kind="ExternalOutput"),
            "dbg_va": nc.dram_tensor("dbg_va", [P, NKC, HG * VW], BF16,
                                     kind="ExternalOutput"),
            "dbg_ex": nc.dram_tensor("dbg_ex", [P, 2, QW], BF16, kind="ExternalOutput"),
            "dbg_r": nc.dram_tensor("dbg_r", [1, QW], F32, kind="ExternalOutput"),
            "dbg_bc": nc.dram_tensor("dbg_bc", [HD, QW], F32, kind="ExternalOutput"),
        }
    with tile.TileContext(nc) as tc:
        _emit(nc, tc, xt_d, wq_d, wk_d, wv_d, wo_d, out_d, dbg)
    nc.compile()
    _NC_CACHE["nc"] = nc
    return nc


def _emit(nc, tc, xt_d, wq_d, wk_d, wv_d, wo_d, out_d, dbg=None):
    with tc.tile_pool(name="big", bufs=1) as big, \
         tc.tile_pool(name="expp", bufs=4) as expp, \
         tc.tile_pool(name="norm", bufs=2) as norm, \
         tc.tile_pool(name="outp", bufs=3) as outp, \
         tc.tile_pool(name="ps_sc", bufs=2, space="PSUM") as ps_sc, \
         tc.tile_pool(name="ps_ctx", bufs=1, space="PSUM") as ps_ctx, \
         tc.tile_pool(name="ps_o", bufs=2, space="PSUM") as ps_o:
        # ---- persistent SBUF tensors (~80KB/partition, bf16) ----
        xs = big.tile([P, KO, S], BF16)         # x.T, [d_in(128) x ko x s]
        wqs = big.tile([P, KO, GC], BF16)
        wks = big.tile([P, KO, GC], BF16)
        wvs = big.tile([P, KO, GC], BF16)
        wos = big.tile([P, 2, D], BF16)         # [d'(128) x chunk x e]
        qt = big.tile([P, 2, S], BF16)          # head h at parts (h%2)*64, chunk h//2
        kt = big.tile([P, 2, S], BF16)
        va = big.tile([P, NKC, HG * VW], BF16)  # V natural + ones col per head
        ct = big.tile([P, 2, S], BF16)          # ctxT, same head layout as qt

        # partition_broadcast runs on the Q7 cores and needs the attn ucode
        # library resident (CoreSim doesn't care, hardware does).
        nc.gpsimd.load_library(library_config.attn)

        # DMA order matters: wk/wq first (KT/QT chains start ASAP), then xs
        # by q-block, V/O weights last (consumed later).
        nc.sync.dma_start(wks[:], wk_d.rearrange("(ko p) m -> p ko m", p=P))
        nc.sync.dma_start(wqs[:], wq_d.rearrange("(ko p) m -> p ko m", p=P))
        xt_r = xt_d.rearrange("(ko p) s -> p ko s", p=P)
        for n in range(NQ):
            nc.sync.dma_start(xs[:, :, n * QW:(n + 1) * QW],
                              xt_r[:, :, n * QW:(n + 1) * QW])
        nc.sync.dma_start(wvs[:], wv_d.rearrange("(ko p) m -> p ko m", p=P))
        nc.sync.dma_start(wos[:], wo_d.rearrange("(c p) e -> p c e", p=P))

        # ones columns of V_aug (col HD of each VW-wide head block)
        va_h = va[:].rearrange("p s (h c) -> p s h c", c=VW)
        for h in range(HG):
            nc.vector.memset(
                va_h[:, :, h, HD:HD + 1].bitcast(mybir.dt.uint16), 0x3F80)

        def proj_chain(w_sb, m, n, dst):
            """dst[:, m, n*QW:] = sum_ko w_sb[:,ko,m*128:+128].T @ xs[:,ko,nq]"""
            pp = ps_o.tile([P, QW], F32, tag="po")
            for ko in range(KO):
                nc.tensor.matmul(pp[:], w_sb[:, ko, m * P:(m + 1) * P],
                                 xs[:, ko, n * QW:(n + 1) * QW],
                                 start=(ko == 0), stop=(ko == KO - 1))
            nc.vector.tensor_copy(dst[:, m, n * QW:(n + 1) * QW], pp[:])

        def v_chain(kc):
            """va[:, kc, heads] = xs[:, :, kc-chunk].T @ wv  (natural V)"""
            pv = ps_o.tile([P, QW], F32, tag="po")
            for ko in range(KO):
                nc.tensor.matmul(pv[:, 0:GC], xs[:, ko, kc * P:(kc + 1) * P],
                                 wvs[:, ko, :],
                                 start=(ko == 0), stop=(ko == KO - 1))
            nc.vector.tensor_copy(
                va_h[:, kc, :, 0:HD],
                pv[:, 0:GC].rearrange("p (h c) -> p h c", c=HD))

        def po_chain(mo, n):
            """out_t[mo*128:+128, nq] = sum_c wos[:,c,mo*128:+128].T @ ct[:,c,nq]"""
            pp = ps_o.tile([P, QW], F32, tag="po")
            for c in range(2):
                nc.tensor.matmul(pp[:], wos[:, c, mo * P:(mo + 1) * P],
                                 ct[:, c, n * QW:(n + 1) * QW],
                                 start=(c == 0), stop=(c == 1))
            ot = outp.tile([P, QW], F32, tag="ot")
            nc.vector.tensor_copy(ot[:], pp[:])
            nc.sync.dma_start(
                out_d[mo * P:(mo + 1) * P, n * QW:(n + 1) * QW], ot[:])

        # ---- lead-in: KT (all), QT for q-blocks 0 and 1 ----
        for m in range(2):
            for n in range(NQ):
                proj_chain(wks, m, n, kt)
        for n in range(2):
            for m in range(2):
                proj_chain(wqs, m, n, qt)

        # ---- main loop: q-block n outer, head-pair hp inner ----
        for n in range(NQ):
            for hp in range(2):
                cps = [ps_ctx.tile([VW, QW], F32, tag=f"pc{e}",
                                   name=f"pc{e}_{hp}_{n}") for e in range(2)]
                for kc in range(NKC):
                    sp = ps_sc.tile([P, 2, QW], F32, tag="psc")
                    for e in range(2):   # head 2hp+e in rows e*64..e*64+63
                        lo = e * HD
                        nc.tensor.matmul(
                            sp[:, e, :],
                            kt[lo:lo + HD, hp, kc * P:(kc + 1) * P],
                            qt[lo:lo + HD, hp, n * QW:(n + 1) * QW],
                            start=True, stop=True)
                    # interleaved fill work to keep the PE dense:
                    if hp == 0:
                        if n == 0:
                            v_chain(kc)           # V projection, needed by ctx
                        elif kc >= 8:
                            po_chain(kc - 8, n - 1)  # output proj of prev q-block
                    else:
                        if n < 2 and kc in (8, 12):  # QT for q-block n+2
                            proj_chain(wqs, (kc - 8) // 4, n + 2, qt)
                    ex = expp.tile([P, 2, QW], BF16, tag="pex")
                    nc.scalar.activation(
                        ex[:].rearrange("p a b -> p (a b)"),
                        sp[:].rearrange("p a b -> p (a b)"),
                        mybir.ActivationFunctionType.Exp,
                        scale=0.125)
                    if dbg is not None and n == 0 and hp == 0 and kc == 0:
                        nc.sync.dma_start(dbg["dbg_ex"][:], ex[:])
                    for e in range(2):
                        h = 2 * hp + e
                        nc.tensor.matmul(
                            cps[e][:],
                            va[:, kc, h * VW:(h + 1) * VW],
                            ex[:, e, :],
                            start=(kc == 0), stop=(kc == NKC - 1))
                # normalize: ctxT = cps[0:64] * (1 / cps[64])
                for e in range(2):
                    # custom-DVE ops drop the input base-partition on HW, so
                    # stage the denominator row to SBUF partition 0 first.
                    dsb = norm.tile([1, QW], F32, tag="nd")
                    nc.vector.tensor_copy(dsb[:], cps[e][HD:VW, :])
                    r = norm.tile([1, QW], F32, tag="nr")
                    nc.vector.reciprocal_approx_fast(r[:], dsb[:])
                    bc = norm.tile([HD, QW], F32, tag="nb")
                    nc.gpsimd.partition_broadcast(bc[:], r[:])
                    if dbg is not None and n == 0 and hp == 0 and e == 0:
                        nc.sync.dma_start(dbg["dbg_r"][:], r[:])
                        nc.sync.dma_start(dbg["dbg_bc"][:], bc[:])
                    nc.vector.tensor_tensor(
                        ct[e * HD:(e + 1) * HD, hp, n * QW:(n + 1) * QW],
                        cps[e][0:HD, :], bc[:], mybir.AluOpType.mult)
        # ---- tail: output projection of the last q-block ----
        for mo in range(KO):
            po_chain(mo, NQ - 1)
        if dbg is not None:
            nc.sync.dma_start(dbg["dbg_kt"][:], kt[:])
            nc.sync.dma_start(dbg["dbg_qt"][:], qt[:])
            nc.sync.dma_start(dbg["dbg_ct"][:], ct[:])
            nc.sync.dma_start(dbg["dbg_va"][:], va[:])


def _in_maps(x, wq_f, wk_f, wv_f, wo_f):
    bf = ml_dtypes.bfloat16
    maps = []
    for core in range(8):
        b, g = core // 4, core % 4
        cols = slice(g * GC, (g + 1) * GC)
        maps.append({
            "xt": np.ascontiguousarray(x[b].T).astype(bf),
            "wq": np.ascontiguousarray(wq_f[:, cols]).astype(bf),
            "wk": np.ascontiguousarray(wk_f[:, cols]).astype(bf),
            "wv": np.ascontiguousarray(wv_f[:, cols]).astype(bf),
            "wo": np.ascontiguousarray(wo_f[cols, :]).astype(bf),
        })
    return maps


def _prep(x, Wq, Wk, Wv, Wo, q_scale, k_scale, v_scale, o_scale):
    x = np.asarray(x, dtype=np.float32)
    wq_f = (np.asarray(Wq).T * np.asarray(q_scale).reshape(1, -1)).astype(np.float32)
    wk_f = (np.asarray(Wk).T * np.asarray(k_scale).reshape(1, -1)).astype(np.float32)
    wv_f = (np.asarray(Wv).T * np.asarray(v_scale).reshape(1, -1)).astype(np.float32)
    wo_f = (np.asarray(Wo).T * np.asarray(o_scale).reshape(1, -1)).astype(np.float32)
    return x, wq_f, wk_f, wv_f, wo_f


def run_traced(x, Wq, Wk, Wv, Wo, q_scale, k_scale, v_scale, o_scale):
    """Like kernel() but with NTFF tracing; returns (out, exec_time_ns, trace_path)."""
    x, wq_f, wk_f, wv_f, wo_f = _prep(x, Wq, Wk, Wv, Wo,
                                      q_scale, k_scale, v_scale, o_scale)
    nc = _build()
    res = run_bass_kernel_spmd(nc, _in_maps(x, wq_f, wk_f, wv_f, wo_f),
                               core_ids=list(range(8)), trace=True)
    out = np.zeros((x.shape[0], S, D), dtype=np.float32)
    for core in range(8):
        out[core // 4] += res.results[core]["out_t"].T
    trace_path = None
    if res.instructions_and_trace is not None:
        trace_path = res.instructions_and_trace[1]
    return out, res.exec_time_ns, trace_path


def kernel(x, Wq, Wk, Wv, Wo, q_scale, k_scale, v_scale, o_scale):
    B = x.shape[0]
    x, wq_f, wk_f, wv_f, wo_f = _prep(x, Wq, Wk, Wv, Wo,
                                      q_scale, k_scale, v_scale, o_scale)
    nc = _build()
    res = run_bass_kernel_spmd(nc, _in_maps(x, wq_f, wk_f, wv_f, wo_f),
                               core_ids=list(range(8)))
    out = np.zeros((B, S, D), dtype=np.float32)
    for core in range(8):
        out[core // 4] += res.results[core]["out_t"].T
    return out
